# revision 10
# baseline (speedup 1.0000x reference)
"""DMN (Dynamic Memory Network) forward pass on 8 Trainium2 NeuronCores.

Data-parallel over batch (16 examples/core). All matmuls run in fp8(e4m3)
with DoubleRow perf mode (two 128-row K-tiles per instruction) accumulating
in fp32 PSUM. States/weights live in a folded-transposed layout
(128 partitions = one 128-row slice of H; free dim = k-tile * width + col).

Tricks:
  - z-gate rows of every GRU weight/bias are negated on the host so one
    sigmoid pass yields [r, 1-z] directly.
  - gate preactivations (gi) and biases are injected into PSUM via identity /
    doubled-bias matmuls, so sigmoid/tanh feeds read PSUM directly (short
    vector chains in the serial episodic loop).
  - inputs arrive pre-gathered AND pre-transposed from the host (no on-device
    DMA transposes).
  - fc (vocab) weights are preloaded into SBUF during the earlier phases in
    fp8; log-softmax runs in two matmul passes (no 32000-wide logits buffer),
    with -log(Z) folded into the final activation bias.

kernel(**inputs) takes FULL unsharded inputs and returns (B*num_decode, V) f32.
"""

import numpy as np
import ml_dtypes

import concourse.bacc as bacc
import concourse.mybir as mybir
import concourse.tile as tile
from concourse import bass_utils

F32 = mybir.dt.float32
BF16 = mybir.dt.bfloat16
FP8 = mybir.dt.float8e4
AF = mybir.ActivationFunctionType
ALU = mybir.AluOpType
DR = mybir.MatmulPerfMode.DoubleRow

H = 512
HQ = 4            # H / 128
G3 = 3 * H
MT = 12           # gate m-tiles
V = 32000
B = 128
NF = 40
L = 12
QL = 16
EPISODES = 3
N_CORES = 8
BC = B // N_CORES
S = BC * NF       # 640 sequences per core
NTQ = BC * QL     # 256
CH = 128          # facts chunk width
NCH = S // CH     # 5
VSLAB = 2048
VSLABS = [VSLAB] * 15 + [V - 15 * VSLAB]
N_FCT_PRELOAD = 10

bf16 = ml_dtypes.bfloat16
f8 = ml_dtypes.float8_e4m3

_COMPILED = {}


class Cfg:
    def __init__(self, bc=BC, nd=4):
        self.bc, self.nd = bc, nd
        self.nv = bc * nd
        self.key = (bc, nd)


def k3(ap_t, kt):
    """(128, kt*X) tile -> (p, kt, X) AP view for k-pair slicing."""
    return ap_t[:].rearrange("p (k x) -> p k x", k=kt)


def build(cfg: Cfg):
    nc = bacc.Bacc("TRN2", target_bir_lowering=False, debug=False,
                   num_devices=N_CORES)
    bn = cfg.bc
    nv = cfg.nv
    nd = cfg.nd

    def din(name, shape, dt=FP8):
        return nc.dram_tensor(name, list(shape), dt, kind="ExternalInput").ap()

    fxT_d = din("fxT", (L * 128, HQ * S))
    qxT_d = din("qxT", (128, HQ * NTQ))
    w_f_ih_d = din("w_f_ih", (128, HQ * G3)); w_f_hh_d = din("w_f_hh", (128, HQ * G3))
    w_q_ih_d = din("w_q_ih", (128, HQ * G3)); w_q_hh_d = din("w_q_hh", (128, HQ * G3))
    w_a_ih_d = din("w_a_ih", (128, HQ * G3)); w_a_hh_d = din("w_a_hh", (128, HQ * G3))
    w_m_ih_d = din("w_m_ih", (128, HQ * G3)); w_m_hh_d = din("w_m_hh", (128, HQ * G3))
    w_ans_ih_d = din("w_ans_ih", (128, 2 * HQ * G3))
    w_ans_hh_d = din("w_ans_hh", (128, HQ * G3))
    g1t_d = din("g1t", (128, 16 * H))
    g2t_d = din("g2t", (128, HQ))
    fct_d = din("fct", (128, HQ * V))
    fcb2_d = din("fcb2", (1, 2 * V))          # per slab: [fcb | zeros]
    y0t_d = din("y0t", (128, HQ))
    i128_d = din("i128", (128, 128), BF16)
    i2f8_d = din("i2f8", (128, 256))          # [I128 | zeros] fp8
    onez_d = din("onez", (1, 128))            # [ones(nv) | zeros] fp8
    brz_f_d = din("brz_f", (128, 8), F32)     # z-cols negated
    bnih_f_d = din("bnih_f", (128, 4), F32)
    bnhh_f_d = din("bnhh_f", (128, 4), F32)
    gib_q_d = din("gib_q", (128, MT), F32)    # cols 4..7 negated
    gib_a_d = din("gib_a", (128, MT), F32)
    gib_ans_d = din("gib_ans", (128, MT), F32)
    bnhh_q_d = din("bnhh_q", (128, 4), F32)
    bnhh_a_d = din("bnhh_a", (128, 4), F32)
    bnhh_ans_d = din("bnhh_ans", (128, 4), F32)
    brz_m_d = din("brz_m", (128, 8), F32)
    bnih_m_d = din("bnih_m", (128, 4), F32)
    bnhh_m_d = din("bnhh_m", (128, 4), F32)
    gb1_d = din("gb1", (128, HQ), F32)
    gb2_d = din("gb2", (1, 1), F32)
    out_d = nc.dram_tensor("out", [nv, V], BF16, kind="ExternalOutput").ap()

    with tile.TileContext(nc) as tc, tc.tile_pool(name="const", bufs=1) as cp:
        frepT = cp.tile([128, HQ * S], FP8, tag="frepT")
        qrepT = cp.tile([128, HQ * bn], FP8, tag="qrepT")
        memT = cp.tile([128, HQ * bn], FP8, tag="memT")
        hans = cp.tile([128, HQ * bn], FP8, tag="hans")
        hdecT = cp.tile([128, HQ * nv], FP8, tag="hdecT")

        def load(pool, ap_d, shape, dt=F32, eng=None):
            t = pool.tile(list(shape), dt, tag=ap_d.tensor.name + "_sb")
            (eng or nc.sync).dma_start(t[:], ap_d[:])
            return t

        i128 = load(cp, i128_d, (128, 128), BF16)
        i2f8 = load(cp, i2f8_d, (128, 256), FP8)
        onez = load(cp, onez_d, (1, 128), FP8)
        y0t = load(cp, y0t_d, (128, HQ), FP8)
        brz_f = load(cp, brz_f_d, (128, 8))
        bnih_f = load(cp, bnih_f_d, (128, 4))
        bnhh_f = load(cp, bnhh_f_d, (128, 4))
        gib_q = load(cp, gib_q_d, (128, MT))
        gib_a = load(cp, gib_a_d, (128, MT))
        gib_ans = load(cp, gib_ans_d, (128, MT))
        bnhh_q = load(cp, bnhh_q_d, (128, 4))
        bnhh_a = load(cp, bnhh_a_d, (128, 4))
        bnhh_ans = load(cp, bnhh_ans_d, (128, 4))
        brz_m = load(cp, brz_m_d, (128, 8))
        bnih_m = load(cp, bnih_m_d, (128, 4))
        bnhh_m = load(cp, bnhh_m_d, (128, 4))
        gb1 = load(cp, gb1_d, (128, HQ))
        gb2 = load(cp, gb2_d, (1, 1))

        # fc-weight slabs: first N_FCT_PRELOAD live in the const pool, DMA'd
        # up front on the idle gpsimd queue; the rest stream through a ring
        fct_sb = []
        off = 0
        for si, vs in enumerate(VSLABS):
            if si < N_FCT_PRELOAD:
                t = cp.tile([128, HQ * VSLAB], FP8, tag=f"fct{si}")
                nc.gpsimd.dma_start(t[:, 0:HQ * vs], fct_d[:, off:off + HQ * vs])
            else:
                t = None
            fct_sb.append((t, off, vs))
            off += HQ * vs

        def bias2(pool, src, cols, rep, tag):
            """Doubled ([bias_bcast | zeros]) fp8 tile for DR PSUM injection."""
            n = cols * rep
            t = pool.tile([128, 2 * n], FP8, tag=tag)
            nc.vector.memset(t[:, n:2 * n], 0.0)
            nc.vector.tensor_copy(
                t[:, 0:n].rearrange("p (c r) -> p c r", c=cols),
                src[:].to_broadcast([128, cols, rep]))
            return t

        bnhhx_q = bias2(cp, bnhh_q, 4, bn, "bnhhx_q")
        bnhhx_a = bias2(cp, bnhh_a, 4, bn, "bnhhx_a")
        bnhhx_ans = bias2(cp, bnhh_ans, 4, bn, "bnhhx_ans")
        brzx_m = bias2(cp, brz_m, 8, bn, "brzx_m")
        bnihx_m = bias2(cp, bnih_m, 4, bn, "bnihx_m")
        bnhhx_m = bias2(cp, bnhh_m, 4, bn, "bnhhx_m")

        def dr_inject(pp_ap, x2, col0, n):
            """psum[:, 0:n] += doubled-bias cols [col0:col0+n] (fp8 DR mms,
            chunked to <=256 cols; chunks stay in one 256-col window)."""
            i2v = k3(i2f8, 2)
            x2v = k3(x2, 2)
            for a in range(0, n, 256):
                w = min(256, n - a)
                nc.tensor.matmul(pp_ap[:, a:a + w], i2v,
                                 x2v[:, :, col0 + a:col0 + a + w],
                                 start=False, stop=True, perf_mode=DR,
                                 skip_group_check=True)

        def gi_inject(pp_ap, gi_ap):
            nc.tensor.matmul(pp_ap, i128[:], gi_ap, start=False, stop=True,
                             skip_group_check=True)

        def mm_dr(tgt, w3, m, src3, start):
            for j in range(2):
                nc.tensor.matmul(tgt, w3[:, 2 * j:2 * j + 2,
                                          m * 128:(m + 1) * 128],
                                 src3[:, 2 * j:2 * j + 2, :],
                                 start=start and j == 0, stop=False,
                                 perf_mode=DR,
                                 skip_group_check=not (start and j == 0))

        def gru_step(sp, pp, ppn, whh3, h, gi_rz, gi_n, bnhhx2, out_ap,
                     gate=None, ih=None, brzx2=None, bnihx2=None, tag=""):
            """One folded GRU step, width bn=16.

            gi_rz (p,8,bn) / gi_n (p,4,bn) bf16 APs, or None with
            ih=(wih3, x3) + brzx2/bnihx2 doubled-bias tiles (mem-GRU path).
            """
            h3 = k3(h, HQ)
            prz = pp.tile([128, 8 * bn], F32, tag="rz")
            for m in range(8):
                tgt = prz[:, m * bn:(m + 1) * bn]
                mm_dr(tgt, whh3, m, h3, True)
                if ih is not None:
                    mm_dr(tgt, ih[0], m, ih[1], False)
            if gi_rz is not None:
                gi_inject(prz[:].rearrange("p (m b) -> p m b", m=8), gi_rz)
            else:
                dr_inject(prz[:], brzx2, 0, 8 * bn)
            rz = sp.tile([128, 8 * bn], BF16, tag="rz" + tag)
            nc.scalar.activation(rz[:], prz[:], AF.Sigmoid)

            pnh = ppn.tile([128, 4 * bn], F32, tag="n")
            for mi in range(4):
                mm_dr(pnh[:, mi * bn:(mi + 1) * bn], whh3, 8 + mi, h3, True)
            dr_inject(pnh[:], bnhhx2, 0, 4 * bn)
            t2 = sp.tile([128, 4 * bn], BF16, tag="t2" + tag)
            nc.vector.tensor_mul(t2[:], pnh[:], rz[:, 0:4 * bn])
            npre = sp.tile([128, 4 * bn], BF16, tag="np" + tag)
            if gi_n is not None:
                nc.vector.tensor_add(
                    npre[:].rearrange("p (m b) -> p m b", m=4),
                    t2[:].rearrange("p (m b) -> p m b", m=4), gi_n)
            else:
                pni = ppn.tile([128, 4 * bn], F32, tag="n")
                for mi in range(4):
                    mm_dr(pni[:, mi * bn:(mi + 1) * bn], ih[0], 8 + mi,
                          ih[1], True)
                dr_inject(pni[:], bnihx2, 0, 4 * bn)
                nc.vector.tensor_add(npre[:], t2[:], pni[:])
            n_t = sp.tile([128, 4 * bn], BF16, tag="nt" + tag)
            nc.scalar.activation(n_t[:], npre[:], AF.Tanh)
            w_ap = rz[:, 4 * bn:8 * bn]
            if gate is not None:
                p_t = sp.tile([128, 4 * bn], BF16, tag="p" + tag)
                nc.vector.tensor_mul(p_t[:], w_ap, gate)
                w_ap = p_t[:]
            d_t = sp.tile([128, 4 * bn], BF16, tag="d" + tag)
            nc.vector.tensor_sub(d_t[:], n_t[:], h[:])
            u_t = sp.tile([128, 4 * bn], BF16, tag="u" + tag)
            nc.vector.tensor_mul(u_t[:], w_ap, d_t[:])
            nc.vector.tensor_add(out_ap, h[:], u_t[:])

        # ================= facts GRU + question GRU =================
        with tc.tile_pool(name="fst", bufs=1) as stp, \
             tc.tile_pool(name="fsp", bufs=2) as sp, \
             tc.tile_pool(name="fpp", bufs=2, space="PSUM") as pp, \
             tc.tile_pool(name="fpn", bufs=2, space="PSUM") as ppn, \
             tc.tile_pool(name="qpr", bufs=1, space="PSUM") as qpr, \
             tc.tile_pool(name="qpn", bufs=1, space="PSUM") as qpn:
            w_f_ih = load(stp, w_f_ih_d, (128, HQ * G3), FP8)
            w_f_hh = load(stp, w_f_hh_d, (128, HQ * G3), FP8)
            w_q_ih = load(stp, w_q_ih_d, (128, HQ * G3), FP8)
            w_q_hh = load(stp, w_q_hh_d, (128, HQ * G3), FP8)
            qxT = load(stp, qxT_d, (128, HQ * NTQ), FP8)
            fxt = []
            for t in range(L):
                ft = stp.tile([128, HQ * S], FP8, tag=f"fxt{t}")
                nc.sync.dma_start(ft[:], fxT_d[t * 128:(t + 1) * 128, :])
                fxt.append(ft)
            brzx8 = bias2(stp, brz_f, 8, CH, "brzx8")
            bnhhx_fc = bias2(stp, bnhh_f, 4, CH, "bnhhx_fc")
            bnihx_fc = bias2(stp, bnih_f, 4, CH, "bnihx_fc")
            giqxb = bias2(stp, gib_q, MT, NTQ, "giqxb")

            wf_ih3, wf_hh3 = k3(w_f_ih, HQ), k3(w_f_hh, HQ)
            wq_ih3, wq_hh3 = k3(w_q_ih, HQ), k3(w_q_hh, HQ)
            qx3 = k3(qxT, HQ)

            # question-GRU input gates: m-major (m, b, t) bf16
            giq = stp.tile([128, MT * NTQ], BF16, tag="giq")
            for mg in range(3):   # 4 m-tiles per psum tile
                psm = pp.tile([128, 4 * NTQ], F32, tag="frz")
                for mi in range(4):
                    mm_dr(psm[:, mi * NTQ:(mi + 1) * NTQ], wq_ih3,
                          mg * 4 + mi, qx3, True)
                dr_inject(psm[:], giqxb, mg * 4 * NTQ, 4 * NTQ)
                nc.scalar.activation(giq[:, mg * 4 * NTQ:(mg + 1) * 4 * NTQ],
                                     psm[:], AF.Identity)
            giq4 = giq[:].rearrange("p (m b t) -> p m b t", m=MT, b=bn)

            hq = stp.tile([128, HQ * bn], FP8, tag="hq")
            nc.vector.memset(hq[:], 0.0)
            hst = []
            for c in range(NCH):
                hc = stp.tile([128, HQ * CH], FP8, tag=f"hf{c}")
                nc.vector.memset(hc[:], 0.0)
                hst.append(hc)

            qstep = [0]

            def q_step():
                t = qstep[0]
                if t >= QL:
                    return
                qstep[0] += 1
                out_ap = qrepT[:] if t == QL - 1 else hq[:]
                gru_step(sp, qpr, qpn, wq_hh3, hq,
                         giq4[:, 0:8, :, t:t + 1].rearrange(
                             "p m b o -> p m (b o)"),
                         giq4[:, 8:12, :, t:t + 1].rearrange(
                             "p m b o -> p m (b o)"),
                         bnhhx_q, out_ap, tag="q")

            nslot = L * NCH
            slot = 0
            for t in range(L):
                x3full = k3(fxt[t], HQ)
                for c in range(NCH):
                    off = c * CH
                    h = hst[c]
                    h3 = k3(h, HQ)
                    x3 = x3full[:, :, off:off + CH]
                    prz = pp.tile([128, 8 * CH], F32, tag="frz")
                    for m in range(8):
                        tgt = prz[:, m * CH:(m + 1) * CH]
                        mm_dr(tgt, wf_ih3, m, x3, True)
                        mm_dr(tgt, wf_hh3, m, h3, False)
                    dr_inject(prz[:], brzx8, 0, 8 * CH)
                    rz = sp.tile([128, 8 * CH], BF16, tag="frzs")
                    nc.scalar.activation(rz[:], prz[:], AF.Sigmoid)

                    pnh = ppn.tile([128, 4 * CH], F32, tag="fn")
                    for mi in range(4):
                        mm_dr(pnh[:, mi * CH:(mi + 1) * CH], wf_hh3, 8 + mi,
                              h3, True)
                    dr_inject(pnh[:], bnhhx_fc, 0, 4 * CH)
                    t2 = sp.tile([128, 4 * CH], BF16, tag="ft2")
                    nc.vector.tensor_mul(t2[:], pnh[:], rz[:, 0:4 * CH])
                    pni = ppn.tile([128, 4 * CH], F32, tag="fn")
                    for mi in range(4):
                        mm_dr(pni[:, mi * CH:(mi + 1) * CH], wf_ih3, 8 + mi,
                              x3, True)
                    dr_inject(pni[:], bnihx_fc, 0, 4 * CH)
                    npre = sp.tile([128, 4 * CH], BF16, tag="fnp")
                    nc.vector.tensor_add(npre[:], t2[:], pni[:])
                    n_t = sp.tile([128, 4 * CH], BF16, tag="fnt")
                    nc.scalar.activation(n_t[:], npre[:], AF.Tanh)
                    d_t = sp.tile([128, 4 * CH], BF16, tag="fd")
                    nc.vector.tensor_sub(d_t[:], n_t[:], h[:])
                    u_t = sp.tile([128, 4 * CH], BF16, tag="fu")
                    nc.vector.tensor_mul(u_t[:], rz[:, 4 * CH:8 * CH], d_t[:])
                    if t == L - 1:
                        out_ap = k3(frepT, HQ)[:, :, off:off + CH]
                        nc.vector.tensor_add(out_ap, k3(h, HQ), k3(u_t, HQ))
                    else:
                        nc.vector.tensor_add(h[:], h[:], u_t[:])
                    slot += 1
                    while qstep[0] * nslot < slot * QL:
                        q_step()
            while qstep[0] < QL:
                q_step()

        # ============ episodic memory + answer + fc ============
        with tc.tile_pool(name="est", bufs=1) as stp, \
             tc.tile_pool(name="esp", bufs=2) as sp:
            w_a_ih = load(stp, w_a_ih_d, (128, HQ * G3), FP8)
            w_a_hh = load(stp, w_a_hh_d, (128, HQ * G3), FP8)
            w_m_ih = load(stp, w_m_ih_d, (128, HQ * G3), FP8)
            w_m_hh = load(stp, w_m_hh_d, (128, HQ * G3), FP8)
            g1t = load(stp, g1t_d, (128, 16 * H), FP8)
            g2t = load(stp, g2t_d, (128, HQ), FP8)
            w_ans_ih = load(stp, w_ans_ih_d, (128, 2 * HQ * G3), FP8)
            w_ans_hh = load(stp, w_ans_hh_d, (128, HQ * G3), FP8)
            wa_ih3, wa_hh3 = k3(w_a_ih, HQ), k3(w_a_hh, HQ)
            wm_ih3, wm_hh3 = k3(w_m_ih, HQ), k3(w_m_hh, HQ)
            wans_ih3, wans_hh3 = k3(w_ans_ih, 2 * HQ), k3(w_ans_hh, HQ)
            g1t3 = k3(g1t, 16)
            g2t3 = g2t[:].rearrange("p (k o) -> p k o", o=1)

            nc.vector.tensor_copy(memT[:], qrepT[:])
            qexp = stp.tile([128, HQ * S], FP8, tag="qexp")
            nc.vector.tensor_copy(
                qexp[:].rearrange("p (qb f) -> p qb f", f=NF),
                qrepT[:].to_broadcast([128, HQ * bn, NF]))
            zfeat = stp.tile([128, 16 * S], FP8, tag="zfeat")
            mexp = stp.tile([128, HQ * S], FP8, tag="mexp")
            gia = stp.tile([128, MT * S], BF16, tag="gia")
            zsc = stp.tile([128, HQ * S], BF16, tag="zsc")
            relu = stp.tile([128, HQ * S], FP8, tag="relu")
            gexc = stp.tile([1, S], BF16, tag="gexc")
            gexb = stp.tile([128, S], BF16, tag="gexb")
            gexp = stp.tile([128, NF * HQ * bn], BF16, tag="gexp")
            gians = stp.tile([128, MT * bn], BF16, tag="gians")
            ansin = stp.tile([128, 2 * HQ * bn], FP8, tag="ansin")

            # episode-invariant zfeat halves: f*q and |f-q|
            nc.vector.tensor_mul(zfeat[:, 0:HQ * S], frepT[:], qexp[:])
            nc.vector.tensor_sub(zsc[:], frepT[:], qexp[:])
            nc.scalar.activation(zfeat[:, 2 * HQ * S:3 * HQ * S], zsc[:],
                                 AF.Abs)

            # attention-GRU input gates gia = Wih_a @ frep + bias, m-major
            with tc.tile_pool(name="giab", bufs=1) as gbp, \
                 tc.tile_pool(name="giap", bufs=2, space="PSUM") as gpp:
                giaxb = bias2(gbp, gib_a, MT, S, "giaxb")
                frep3 = k3(frepT, HQ)
                total = MT * S
                gt0 = 0
                while gt0 < total:
                    gw = min(1024, total - gt0)
                    psm = gpp.tile([128, 1024], F32, tag="giapp")
                    a0 = gt0
                    while a0 < gt0 + gw:
                        m = a0 // S
                        a1 = min((m + 1) * S, gt0 + gw,
                                 256 * (a0 // 256 + 1))
                        tgt = psm[:, a0 - gt0:a1 - gt0]
                        for j in range(2):
                            nc.tensor.matmul(
                                tgt, wa_ih3[:, 2 * j:2 * j + 2,
                                            m * 128:(m + 1) * 128],
                                frep3[:, 2 * j:2 * j + 2,
                                      a0 - m * S:a1 - m * S],
                                start=j == 0, stop=False, perf_mode=DR,
                                skip_group_check=j != 0)
                        a0 = a1
                    dr_inject(psm[:, 0:gw], giaxb, gt0, gw)
                    nc.scalar.activation(gia[:, gt0:gt0 + gw],
                                         psm[:, 0:gw], AF.Identity)
                    gt0 += gw
            gia4 = gia[:].rearrange("p (m b f) -> p m b f", m=MT, b=bn)

            # answer-GRU input gates (ans_in is constant across decode steps)
            nc.vector.tensor_copy(
                ansin[:, 0:HQ * bn].rearrange("p (q b) -> p q b", q=HQ),
                y0t[:].to_broadcast([128, HQ, bn]))
            nc.vector.tensor_copy(ansin[:, HQ * bn:2 * HQ * bn], qrepT[:])

            with tc.tile_pool(name="epz", bufs=2, space="PSUM") as pz, \
                 tc.tile_pool(name="epn", bufs=2, space="PSUM") as pn, \
                 tc.tile_pool(name="eg1", bufs=1, space="PSUM") as pg1, \
                 tc.tile_pool(name="eg2", bufs=2, space="PSUM") as pg2:
                # gians: all 12 m-tiles in one psum tile
                giansxb = bias2(stp, gib_ans, MT, bn, "giansxb")
                ans3 = k3(ansin, 2 * HQ)
                psm = pz.tile([128, MT * bn], F32, tag="rz")
                for m in range(MT):
                    tgt = psm[:, m * bn:(m + 1) * bn]
                    for j in range(4):
                        nc.tensor.matmul(tgt, wans_ih3[:, 2 * j:2 * j + 2,
                                                       m * 128:(m + 1) * 128],
                                         ans3[:, 2 * j:2 * j + 2, :],
                                         start=j == 0, stop=False,
                                         perf_mode=DR,
                                         skip_group_check=j != 0)
                dr_inject(psm[:], giansxb, 0, MT * bn)
                nc.scalar.activation(gians[:], psm[:], AF.Identity)
                gians3 = gians[:].rearrange("p (m b) -> p m b", m=MT)

                he = stp.tile([128, HQ * bn], FP8, tag="he")
                for e in range(EPISODES):
                    # memory-dependent zfeat halves
                    nc.vector.tensor_copy(
                        mexp[:].rearrange("p (qb f) -> p qb f", f=NF),
                        memT[:].to_broadcast([128, HQ * bn, NF]))
                    nc.vector.tensor_mul(zfeat[:, HQ * S:2 * HQ * S],
                                         frepT[:], mexp[:])
                    nc.vector.tensor_sub(zsc[:], frepT[:], mexp[:])
                    nc.scalar.activation(zfeat[:, 3 * HQ * S:4 * HQ * S],
                                         zsc[:], AF.Abs)
                    zf3 = k3(zfeat, 16)
                    # gate MLP layer 1: relu(zfeat @ g_w1.T + b1)
                    for m in range(HQ):
                        psg = pg1.tile([128, 1024], F32, tag="g1p")
                        b0 = 0
                        for nb in (256, 256, 128):
                            tgt = psg[:, b0:b0 + nb]
                            for j in range(8):
                                nc.tensor.matmul(
                                    tgt, g1t3[:, 2 * j:2 * j + 2,
                                              m * 128:(m + 1) * 128],
                                    zf3[:, 2 * j:2 * j + 2, b0:b0 + nb],
                                    start=j == 0, stop=j == 7, perf_mode=DR,
                                    skip_group_check=j != 0)
                            b0 += nb
                        nc.scalar.activation(relu[:, m * S:(m + 1) * S],
                                             psg[:, 0:S], AF.Relu,
                                             bias=gb1[:, m:m + 1])
                    # gate MLP layer 2 + sigmoid
                    relu3 = k3(relu, HQ)
                    b0 = 0
                    for nb in (256, 256, 128):
                        psg2 = pg2.tile([1, 256], F32, tag="g2p")
                        for j in range(HQ):
                            nc.tensor.matmul(psg2[0:1, 0:nb],
                                             g2t[:, j:j + 1],
                                             relu3[:, j, b0:b0 + nb],
                                             start=j == 0, stop=j == HQ - 1)
                        nc.scalar.activation(gexc[0:1, b0:b0 + nb],
                                             psg2[0:1, 0:nb], AF.Sigmoid,
                                             bias=gb2[:])
                        b0 += nb
                    nc.gpsimd.partition_broadcast(gexb[:], gexc[0:1, :])
                    # expand gates to (t, q, b) layout
                    nc.vector.tensor_copy(
                        gexp[:].rearrange("p (t q b) -> p t q b", t=NF, q=HQ),
                        gexb[:].rearrange("p (b t o) -> p t o b", o=1, b=bn)
                        .to_broadcast([128, NF, HQ, bn]))

                    nc.vector.memset(he[:], 0.0)
                    for t in range(NF):
                        gru_step(
                            sp, pz, pn, wa_hh3, he,
                            gia4[:, 0:8, :, t:t + 1].rearrange(
                                "p m b o -> p m (b o)"),
                            gia4[:, 8:12, :, t:t + 1].rearrange(
                                "p m b o -> p m (b o)"),
                            bnhhx_a, he[:],
                            gate=gexp[:, t * HQ * bn:(t + 1) * HQ * bn],
                            tag="e")
                    # memory GRU
                    gru_step(sp, pz, pn, wm_hh3, memT, None, None, bnhhx_m,
                             memT[:], ih=(wm_ih3, k3(he, HQ)), brzx2=brzx_m,
                             bnihx2=bnihx_m, tag="m")

                # ---- answer decode ----
                hd4 = hdecT[:].rearrange("p (q b dd) -> p q b dd", q=HQ, b=bn)
                hcur = memT
                for di in range(nd):
                    gru_step(sp, pz, pn, wans_hh3, hcur,
                             gians3[:, 0:8, :], gians3[:, 8:12, :],
                             bnhhx_ans, hans[:], tag="a")
                    hcur = hans
                    nc.vector.tensor_copy(
                        hd4[:, :, :, di:di + 1],
                        hans[:].rearrange("p (q b) -> p q b",
                                          q=HQ).to_broadcast([128, HQ, bn, 1]))

        # ---- fc + log-softmax (two matmul passes) ----
        with tc.tile_pool(name="fcst", bufs=1) as stp, \
             tc.tile_pool(name="fcw", bufs=2) as fwp, \
             tc.tile_pool(name="fco", bufs=2) as fop, \
             tc.tile_pool(name="fcp", bufs=2, space="PSUM") as fpp:
                hdec3 = k3(hdecT, HQ)
                onez3 = onez[0:1, :].rearrange("p (k m) -> p k m", k=2)
                sums = stp.tile([nv, len(VSLABS)], F32, tag="sums")

                def slab_tile(si, eng):
                    t, soff, vs = fct_sb[si]
                    if t is not None:
                        return t
                    st = fwp.tile([128, HQ * VSLAB], FP8, tag="fcts")
                    eng.dma_start(st[:, 0:HQ * vs],
                                  fct_d[:, soff:soff + HQ * vs])
                    return st

                def fc_mms(si, wt, eng):
                    _, soff, vs = fct_sb[si]
                    voff = soff // HQ
                    psm = fpp.tile([nv, VSLAB], F32, tag="fcp")
                    w3 = wt[:, 0:HQ * vs].rearrange("p (k v) -> p k v", k=HQ)
                    nsb = (vs + 255) // 256
                    for i in range(nsb):
                        i0, i1 = i * 256, min((i + 1) * 256, vs)
                        tgt = psm[:, i0:i1]
                        for j in range(2):
                            nc.tensor.matmul(tgt, hdec3[:, 2 * j:2 * j + 2, :],
                                             w3[:, 2 * j:2 * j + 2, i0:i1],
                                             start=j == 0, stop=False,
                                             perf_mode=DR,
                                             skip_group_check=j != 0)
                    fcbs = fop.tile([1, 2 * VSLAB], FP8, tag="fcbs")
                    eng.dma_start(fcbs[0:1, 0:2 * vs],
                                  fcb2_d[0:1, 2 * voff:2 * voff + 2 * vs])
                    for i in range(nsb):
                        i0, i1 = i * 256, min((i + 1) * 256, vs)
                        nc.tensor.matmul(psm[:, i0:i1], onez[0:1, 0:nv],
                                         fcbs[0:1, i0:i1], start=False,
                                         stop=True, skip_group_check=True)
                    return psm, vs, voff

                for si in range(len(VSLABS)):
                    wt = slab_tile(si, nc.gpsimd)
                    psm, vs, voff = fc_mms(si, wt, nc.gpsimd)
                    ex = fop.tile([nv, VSLAB], BF16, tag="ex")
                    nc.scalar.activation(ex[:, 0:vs], psm[:, 0:vs], AF.Exp,
                                         accum_out=sums[:, si:si + 1])
                ssum = stp.tile([nv, 1], F32, tag="ssum")
                nc.vector.reduce_sum(ssum[:], sums[:],
                                     axis=mybir.AxisListType.X)
                logz = stp.tile([nv, 1], F32, tag="logz")
                nc.scalar.activation(logz[:], ssum[:], AF.Ln)
                nlz = stp.tile([nv, 1], F32, tag="nlz")
                nc.vector.tensor_scalar(nlz[:], logz[:], -1.0, None, ALU.mult)

                for si in range(len(VSLABS)):
                    wt = slab_tile(si, nc.sync)
                    psm, vs, voff = fc_mms(si, wt, nc.sync)
                    stage = fop.tile([nv, VSLAB], BF16, tag="stage")
                    nc.scalar.activation(stage[:, 0:vs], psm[:, 0:vs],
                                         AF.Identity, bias=nlz[:, 0:1])
                    nc.sync.dma_start(out_d[:, voff:voff + vs],
                                      stage[:, 0:vs])

    nc.compile()
    return nc


def host_prep(inputs, cfg: Cfg):
    bc, nd = cfg.bc, cfg.nd
    emb = np.asarray(inputs["emb"], np.float32).copy()
    emb[0] = 0.0
    facts = np.asarray(inputs["facts"])
    questions = np.asarray(inputs["questions"])
    b = facts.shape[0]
    ncores = b // bc

    flens = (np.asarray(inputs["facts_mask"]).reshape(b * NF, L) == 0).sum(-1)
    qlens = (np.asarray(inputs["question_masks"]) == 0).sum(-1)
    assert (flens == L).all() and (qlens == QL).all(), \
        "kernel requires full-length sequences (masks all zero)"

    ii = {k: np.asarray(vv, np.float32) for k, vv in inputs.items()
          if k not in ("facts", "facts_mask", "questions", "question_masks",
                       "num_decode")}

    def negz(w):
        w = w.copy()
        w[H:2 * H] *= -1.0
        return w

    def wt_tiles(w, kt):
        wt = w.T.reshape(kt, 128, w.shape[0]).transpose(1, 0, 2)
        return np.ascontiguousarray(wt).reshape(128, kt * w.shape[0]).astype(f8)

    def col_tiles(x, ncol):
        return np.ascontiguousarray(x.reshape(ncol, 128).T).astype(np.float32)

    shared = {}
    for nm, wih, whh in (("f", "ig_Wih", "ig_Whh"), ("q", "qg_Wih", "qg_Whh"),
                         ("a", "a_Wih", "a_Whh"), ("m", "m_Wih", "m_Whh")):
        shared[f"w_{nm}_ih"] = wt_tiles(negz(ii[wih]), HQ)
        shared[f"w_{nm}_hh"] = wt_tiles(negz(ii[whh]), HQ)
    shared["w_ans_ih"] = wt_tiles(negz(ii["ans_Wih"]), 2 * HQ)
    shared["w_ans_hh"] = wt_tiles(negz(ii["ans_Whh"]), HQ)
    g1 = ii["g_w1"].T  # (4H, H)
    shared["g1t"] = np.ascontiguousarray(
        g1.reshape(16, 128, H).transpose(1, 0, 2)).reshape(
            128, 16 * H).astype(f8)
    shared["g2t"] = col_tiles(ii["g_w2"][0], HQ).astype(f8)

    fcw = ii["fc_w"]
    fct_full = np.ascontiguousarray(
        fcw.T.reshape(HQ, 128, V).transpose(1, 0, 2))  # (128, HQ, V)
    slabs = []
    off = 0
    for vs in VSLABS:
        slabs.append(fct_full[:, :, off:off + vs].reshape(128, HQ * vs))
        off += vs
    shared["fct"] = np.concatenate(slabs, axis=1).astype(f8)
    fcb = ii["fc_b"]
    fcb2 = []
    off = 0
    for vs in VSLABS:
        fcb2.append(fcb[off:off + vs])
        fcb2.append(np.zeros(vs, np.float32))
        off += vs
    shared["fcb2"] = np.concatenate(fcb2).reshape(1, 2 * V).astype(f8)
    shared["y0t"] = col_tiles(emb[1], HQ).astype(f8)
    shared["i128"] = np.eye(128, dtype=np.float32).astype(bf16)
    i2 = np.zeros((128, 256), np.float32)
    i2[:, 0:128] = np.eye(128)
    shared["i2f8"] = i2.astype(f8)
    onez = np.zeros((1, 128), np.float32)
    onez[0, 0:cfg.nv] = 1.0
    shared["onez"] = onez.astype(f8)

    def neg_rz(v):
        v = v.copy()
        v[H:2 * H] *= -1.0
        return v

    for nm, bih, bhh in (("f", "ig_bih", "ig_bhh"), ("q", "qg_bih", "qg_bhh"),
                         ("a", "a_bih", "a_bhh"), ("m", "m_bih", "m_bhh"),
                         ("ans", "ans_bih", "ans_bhh")):
        bi, bh = ii[bih], ii[bhh]
        if nm in ("q", "a", "ans"):
            gib = np.concatenate([neg_rz(bi + bh)[0:2 * H], bi[2 * H:3 * H]])
            shared[f"gib_{nm}"] = col_tiles(gib, MT)
        shared[f"bnhh_{nm}"] = col_tiles(bh[2 * H:3 * H], 4)
        if nm in ("f", "m"):
            shared[f"brz_{nm}"] = col_tiles(neg_rz(bi + bh)[0:2 * H], 8)
            shared[f"bnih_{nm}"] = col_tiles(bi[2 * H:3 * H], 4)
    shared["gb1"] = col_tiles(ii["g_b1"], HQ)
    shared["gb2"] = ii["g_b2"].reshape(1, 1).astype(np.float32)

    # inputs: pre-gathered + pre-transposed, fp8
    fx = emb[facts.reshape(-1)].reshape(b * NF, L, HQ, 128).astype(f8)
    qx = emb[questions.reshape(-1)].reshape(b, QL, HQ, 128).astype(f8)

    in_maps = []
    for c in range(ncores):
        m = dict(shared)
        a = fx[c * bc * NF:(c + 1) * bc * NF]          # (S, L, HQ, 128)
        m["fxT"] = np.ascontiguousarray(
            a.transpose(1, 3, 2, 0)).reshape(L * 128, HQ * S)
        qa = qx[c * bc:(c + 1) * bc].reshape(bc * QL, HQ, 128)
        m["qxT"] = np.ascontiguousarray(
            qa.transpose(2, 1, 0)).reshape(128, HQ * NTQ)
        in_maps.append(m)
    return in_maps


def kernel(**inputs):
    nd = int(np.asarray(inputs["num_decode"]))
    cfg = Cfg(nd=nd)
    if cfg.key not in _COMPILED:
        _COMPILED[cfg.key] = build(cfg)
    nc = _COMPILED[cfg.key]
    in_maps = host_prep(inputs, cfg)
    res = bass_utils.run_bass_kernel_spmd(nc, in_maps,
                                          core_ids=list(range(N_CORES)))
    out = np.concatenate([res.results[c]["out"] for c in range(N_CORES)], 0)
    return np.ascontiguousarray(out.astype(np.float32))


# revision 15
# speedup vs baseline: 1.1143x; 1.1143x over previous
"""DMN (Dynamic Memory Network) forward pass on 8 Trainium2 NeuronCores.

Data-parallel over batch (16 examples/core). All matmuls run in fp8(e4m3)
with DoubleRow perf mode (two 128-row K-tiles per instruction) accumulating
in fp32 PSUM. States/weights live in a folded-transposed layout
(128 partitions = one 128-row slice of H; free dim = k-tile * width + col).

Tricks:
  - z-gate rows of every GRU weight/bias are negated on the host so one
    sigmoid pass yields [r, 1-z] directly.
  - gate preactivations (gi) and biases are injected into PSUM via identity /
    doubled-bias matmuls, so sigmoid/tanh feeds read PSUM directly (short
    vector chains in the serial episodic loop).
  - inputs arrive pre-gathered AND pre-transposed from the host (no on-device
    DMA transposes).
  - fc (vocab) weights are preloaded into SBUF during the earlier phases in
    fp8; log-softmax runs in two matmul passes (no 32000-wide logits buffer),
    with -log(Z) folded into the final activation bias.

kernel(**inputs) takes FULL unsharded inputs and returns (B*num_decode, V) f32.
"""

import numpy as np
import ml_dtypes

import concourse.bacc as bacc
import concourse.mybir as mybir
import concourse.tile as tile
from concourse import bass_utils

F32 = mybir.dt.float32
BF16 = mybir.dt.bfloat16
FP8 = mybir.dt.float8e4
AF = mybir.ActivationFunctionType
ALU = mybir.AluOpType
DR = mybir.MatmulPerfMode.DoubleRow

H = 512
HQ = 4            # H / 128
G3 = 3 * H
MT = 12           # gate m-tiles
V = 32000
B = 128
NF = 40
L = 12
QL = 16
EPISODES = 3
N_CORES = 8
BC = B // N_CORES
S = BC * NF       # 640 sequences per core
NTQ = BC * QL     # 256
CH = 128          # facts chunk width
NCH = S // CH     # 5
VSLAB = 2048
VSLABS = [VSLAB] * 15 + [V - 15 * VSLAB]
N_FCT_PRELOAD = 10

bf16 = ml_dtypes.bfloat16
f8 = ml_dtypes.float8_e4m3

_COMPILED = {}
DEBUG_DUMP = False


class Cfg:
    def __init__(self, bc=BC, nd=4):
        self.bc, self.nd = bc, nd
        self.nv = bc * nd
        self.key = (bc, nd)


def k3(ap_t, kt):
    """(128, kt*X) tile -> (p, kt, X) AP view for k-pair slicing."""
    return ap_t[:].rearrange("p (k x) -> p k x", k=kt)


def build(cfg: Cfg):
    nc = bacc.Bacc("TRN2", target_bir_lowering=False, debug=False,
                   num_devices=N_CORES)
    bn = cfg.bc
    nv = cfg.nv
    nd = cfg.nd

    def din(name, shape, dt=FP8):
        return nc.dram_tensor(name, list(shape), dt, kind="ExternalInput").ap()

    fxT_d = din("fxT", (L * 128, HQ * S))
    qxT_d = din("qxT", (128, HQ * NTQ))
    w_f_ih_d = din("w_f_ih", (128, HQ * G3)); w_f_hh_d = din("w_f_hh", (128, HQ * G3))
    w_q_ih_d = din("w_q_ih", (128, HQ * G3)); w_q_hh_d = din("w_q_hh", (128, HQ * G3))
    w_a_ih_d = din("w_a_ih", (128, HQ * G3)); w_a_hh_d = din("w_a_hh", (128, HQ * G3))
    w_m_ih_d = din("w_m_ih", (128, HQ * G3)); w_m_hh_d = din("w_m_hh", (128, HQ * G3))
    w_ans_ih_d = din("w_ans_ih", (128, 2 * HQ * G3))
    w_ans_hh_d = din("w_ans_hh", (128, HQ * G3))
    g1t_d = din("g1t", (128, 16 * H))
    g2t_d = din("g2t", (128, HQ))
    fct_d = din("fct", (128, HQ * V))
    fcb2_d = din("fcb2", (1, 2 * V))          # per slab: [fcb | zeros]
    y0t_d = din("y0t", (128, HQ))
    i128_d = din("i128", (128, 128), BF16)
    i2f8_d = din("i2f8", (128, 256))          # [I128 | zeros] fp8
    onez_d = din("onez", (1, 128))            # [ones(nv) | zeros] fp8
    brz_f_d = din("brz_f", (128, 8), F32)     # z-cols negated
    bnih_f_d = din("bnih_f", (128, 4), F32)
    bnhh_f_d = din("bnhh_f", (128, 4), F32)
    gib_q_d = din("gib_q", (128, MT), F32)    # cols 4..7 negated
    gib_a_d = din("gib_a", (128, MT), F32)
    gib_ans_d = din("gib_ans", (128, MT), F32)
    bnhh_q_d = din("bnhh_q", (128, 4), F32)
    bnhh_a_d = din("bnhh_a", (128, 4), F32)
    bnhh_ans_d = din("bnhh_ans", (128, 4), F32)
    brz_m_d = din("brz_m", (128, 8), F32)
    bnih_m_d = din("bnih_m", (128, 4), F32)
    bnhh_m_d = din("bnhh_m", (128, 4), F32)
    gb1_d = din("gb1", (128, HQ), F32)
    gb2_d = din("gb2", (1, 1), F32)
    out_d = nc.dram_tensor("out", [nv, V], BF16, kind="ExternalOutput").ap()
    dbg = {}
    if DEBUG_DUMP:
        for nm, shape in (("d_qrep", (128, HQ * bn)), ("d_frep", (128, HQ * S)),
                          ("d_mem", (128, HQ * bn)), ("d_gex", (128, S)),
                          ("d_hdec", (128, HQ * nv)), ("d_giq", (128, MT * NTQ)),
                          ("d_gia", (128, MT * S))):
            dbg[nm] = nc.dram_tensor(nm, list(shape), F32,
                                     kind="ExternalOutput").ap()

    with tile.TileContext(nc) as tc, tc.tile_pool(name="const", bufs=1) as cp:
        frepT = cp.tile([128, HQ * S], FP8, tag="frepT")
        qrepT = cp.tile([128, HQ * bn], FP8, tag="qrepT")
        memT = cp.tile([128, HQ * bn], FP8, tag="memT")
        hans = cp.tile([128, HQ * bn], FP8, tag="hans")
        hdecT = cp.tile([128, HQ * nv], FP8, tag="hdecT")

        def load(pool, ap_d, shape, dt=F32, eng=None):
            t = pool.tile(list(shape), dt, tag=ap_d.tensor.name + "_sb")
            (eng or nc.sync).dma_start(t[:], ap_d[:])
            return t

        i128 = load(cp, i128_d, (128, 128), BF16)
        i2f8 = load(cp, i2f8_d, (128, 256), FP8)
        onez = load(cp, onez_d, (1, 128), FP8)
        y0t = load(cp, y0t_d, (128, HQ), FP8)
        brz_f = load(cp, brz_f_d, (128, 8))
        bnih_f = load(cp, bnih_f_d, (128, 4))
        bnhh_f = load(cp, bnhh_f_d, (128, 4))
        gib_q = load(cp, gib_q_d, (128, MT))
        gib_a = load(cp, gib_a_d, (128, MT))
        gib_ans = load(cp, gib_ans_d, (128, MT))
        bnhh_q = load(cp, bnhh_q_d, (128, 4))
        bnhh_a = load(cp, bnhh_a_d, (128, 4))
        bnhh_ans = load(cp, bnhh_ans_d, (128, 4))
        brz_m = load(cp, brz_m_d, (128, 8))
        bnih_m = load(cp, bnih_m_d, (128, 4))
        bnhh_m = load(cp, bnhh_m_d, (128, 4))
        gb1 = load(cp, gb1_d, (128, HQ))
        gb2 = load(cp, gb2_d, (1, 1))

        # fc-weight slabs: first N_FCT_PRELOAD live in the const pool, DMA'd
        # up front on the idle gpsimd queue; the rest stream through a ring
        fct_sb = []
        off = 0
        for si, vs in enumerate(VSLABS):
            if si < N_FCT_PRELOAD:
                t = cp.tile([128, HQ * VSLAB], FP8, tag=f"fct{si}")
                nc.gpsimd.dma_start(t[:, 0:HQ * vs], fct_d[:, off:off + HQ * vs])
            else:
                t = None
            fct_sb.append((t, off, vs))
            off += HQ * vs

        def bias2(pool, src, cols, rep, tag):
            """Doubled ([bias_bcast | zeros]) fp8 tile for DR PSUM injection."""
            n = cols * rep
            t = pool.tile([128, 2 * n], FP8, tag=tag)
            nc.vector.memset(t[:, n:2 * n], 0.0)
            nc.vector.tensor_copy(
                t[:, 0:n].rearrange("p (c r) -> p c r", c=cols),
                src[:].to_broadcast([128, cols, rep]))
            return t

        bnhhx_q = bias2(cp, bnhh_q, 4, bn, "bnhhx_q")
        bnhhx_a = bias2(cp, bnhh_a, 4, bn, "bnhhx_a")
        bnhhx_ans = bias2(cp, bnhh_ans, 4, bn, "bnhhx_ans")
        brzx_m = bias2(cp, brz_m, 8, bn, "brzx_m")
        bnihx_m = bias2(cp, bnih_m, 4, bn, "bnihx_m")
        bnhhx_m = bias2(cp, bnhh_m, 4, bn, "bnhhx_m")

        def dr_inject(pp_ap, x2, col0, n):
            """psum[:, 0:n] += doubled-bias cols [col0:col0+n] (fp8 DR mms,
            chunked to <=256 cols; chunks stay in one 256-col window)."""
            i2v = k3(i2f8, 2)
            x2v = k3(x2, 2)
            for a in range(0, n, 256):
                w = min(256, n - a)
                nc.tensor.matmul(pp_ap[:, a:a + w], i2v,
                                 x2v[:, :, col0 + a:col0 + a + w],
                                 start=False, stop=True, perf_mode=DR,
                                 skip_group_check=True)

        def gi_inject(pp_ap, gi_ap):
            nc.tensor.matmul(pp_ap, i128[:], gi_ap, start=False, stop=True,
                             skip_group_check=True)

        def dump(pool, nm, src_ap, shape):
            if not DEBUG_DUMP:
                return
            st = pool.tile(list(shape), F32, tag="dump_" + nm)
            nc.vector.tensor_copy(st[:], src_ap)
            nc.sync.dma_start(dbg[nm][:], st[:])

        def mm_dr(tgt, w3, m, src3, start):
            # start means: this is the first matmul issued into this PSUM
            # BANK (hardware zeroes the whole 2KB bank on start=True)
            for j in range(2):
                nc.tensor.matmul(tgt, w3[:, 2 * j:2 * j + 2,
                                          m * 128:(m + 1) * 128],
                                 src3[:, 2 * j:2 * j + 2, :],
                                 start=start and j == 0, stop=False,
                                 perf_mode=DR,
                                 skip_group_check=not (start and j == 0))

        def gru_step(sp, pp, ppn, whh3, h, gi_rz, gi_n, bnhhx2, out_ap,
                     gate=None, ih=None, brzx2=None, bnihx2=None, tag=""):
            """One folded GRU step, width bn=16.

            gi_rz (p,8,bn) / gi_n (p,4,bn) bf16 APs, or None with
            ih=(wih3, x3) + brzx2/bnihx2 doubled-bias tiles (mem-GRU path).
            """
            h3 = k3(h, HQ)
            prz = pp.tile([128, 8 * bn], F32, tag="rz")
            for m in range(8):
                tgt = prz[:, m * bn:(m + 1) * bn]
                mm_dr(tgt, whh3, m, h3, m == 0)
                if ih is not None:
                    mm_dr(tgt, ih[0], m, ih[1], False)
            if gi_rz is not None:
                gi_inject(prz[:].rearrange("p (m b) -> p m b", m=8), gi_rz)
            else:
                dr_inject(prz[:], brzx2, 0, 8 * bn)
            rz = sp.tile([128, 8 * bn], BF16, tag="rz" + tag)
            nc.scalar.activation(rz[:], prz[:], AF.Sigmoid)

            pnh = ppn.tile([128, 4 * bn], F32, tag="n")
            for mi in range(4):
                mm_dr(pnh[:, mi * bn:(mi + 1) * bn], whh3, 8 + mi, h3,
                      mi == 0)
            dr_inject(pnh[:], bnhhx2, 0, 4 * bn)
            t2 = sp.tile([128, 4 * bn], BF16, tag="t2" + tag)
            nc.vector.tensor_mul(t2[:], pnh[:], rz[:, 0:4 * bn])
            npre = sp.tile([128, 4 * bn], BF16, tag="np" + tag)
            if gi_n is not None:
                nc.vector.tensor_add(
                    npre[:].rearrange("p (m b) -> p m b", m=4),
                    t2[:].rearrange("p (m b) -> p m b", m=4), gi_n)
            else:
                pni = ppn.tile([128, 4 * bn], F32, tag="n")
                for mi in range(4):
                    mm_dr(pni[:, mi * bn:(mi + 1) * bn], ih[0], 8 + mi,
                          ih[1], mi == 0)
                dr_inject(pni[:], bnihx2, 0, 4 * bn)
                nc.vector.tensor_add(npre[:], t2[:], pni[:])
            n_t = sp.tile([128, 4 * bn], BF16, tag="nt" + tag)
            nc.scalar.activation(n_t[:], npre[:], AF.Tanh)
            w_ap = rz[:, 4 * bn:8 * bn]
            if gate is not None:
                p_t = sp.tile([128, 4 * bn], BF16, tag="p" + tag)
                nc.vector.tensor_mul(p_t[:], w_ap, gate)
                w_ap = p_t[:]
            d_t = sp.tile([128, 4 * bn], BF16, tag="d" + tag)
            nc.vector.tensor_sub(d_t[:], n_t[:], h[:])
            u_t = sp.tile([128, 4 * bn], BF16, tag="u" + tag)
            nc.vector.tensor_mul(u_t[:], w_ap, d_t[:])
            nc.vector.tensor_add(out_ap, h[:], u_t[:])

        # ================= facts GRU + question GRU =================
        with tc.tile_pool(name="fst", bufs=1) as stp, \
             tc.tile_pool(name="fsp", bufs=2) as sp, \
             tc.tile_pool(name="fpp", bufs=2, space="PSUM") as pp, \
             tc.tile_pool(name="fpn", bufs=2, space="PSUM") as ppn, \
             tc.tile_pool(name="qpr", bufs=1, space="PSUM") as qpr, \
             tc.tile_pool(name="qpn", bufs=1, space="PSUM") as qpn:
            w_f_ih = load(stp, w_f_ih_d, (128, HQ * G3), FP8)
            w_f_hh = load(stp, w_f_hh_d, (128, HQ * G3), FP8)
            w_q_ih = load(stp, w_q_ih_d, (128, HQ * G3), FP8)
            w_q_hh = load(stp, w_q_hh_d, (128, HQ * G3), FP8)
            qxT = load(stp, qxT_d, (128, HQ * NTQ), FP8)
            fxt = []
            for t in range(L):
                ft = stp.tile([128, HQ * S], FP8, tag=f"fxt{t}")
                nc.sync.dma_start(ft[:], fxT_d[t * 128:(t + 1) * 128, :])
                fxt.append(ft)
            brzx8 = bias2(stp, brz_f, 8, CH, "brzx8")
            bnhhx_fc = bias2(stp, bnhh_f, 4, CH, "bnhhx_fc")
            bnihx_fc = bias2(stp, bnih_f, 4, CH, "bnihx_fc")
            giqxb = bias2(stp, gib_q, MT, NTQ, "giqxb")

            wf_ih3, wf_hh3 = k3(w_f_ih, HQ), k3(w_f_hh, HQ)
            wq_ih3, wq_hh3 = k3(w_q_ih, HQ), k3(w_q_hh, HQ)
            qx3 = k3(qxT, HQ)

            # question-GRU input gates: m-major (m, b, t) bf16
            giq = stp.tile([128, MT * NTQ], BF16, tag="giq")
            for mg in range(3):   # 4 m-tiles per psum tile
                psm = pp.tile([128, 4 * NTQ], F32, tag="frz")
                for mi in range(4):
                    mm_dr(psm[:, mi * NTQ:(mi + 1) * NTQ], wq_ih3,
                          mg * 4 + mi, qx3, mi in (0, 2))
                dr_inject(psm[:], giqxb, mg * 4 * NTQ, 4 * NTQ)
                nc.scalar.activation(giq[:, mg * 4 * NTQ:(mg + 1) * 4 * NTQ],
                                     psm[:], AF.Identity)
            giq4 = giq[:].rearrange("p (m b t) -> p m b t", m=MT, b=bn)
            dump(stp, "d_giq", giq[:], (128, MT * NTQ))

            hq = stp.tile([128, HQ * bn], FP8, tag="hq")
            nc.vector.memset(hq[:], 0.0)
            hst = []
            for c in range(NCH):
                hc = stp.tile([128, HQ * CH], FP8, tag=f"hf{c}")
                nc.vector.memset(hc[:], 0.0)
                hst.append(hc)

            qstep = [0]

            def q_step():
                t = qstep[0]
                if t >= QL:
                    return
                qstep[0] += 1
                out_ap = qrepT[:] if t == QL - 1 else hq[:]
                gru_step(sp, qpr, qpn, wq_hh3, hq,
                         giq4[:, 0:8, :, t:t + 1].rearrange(
                             "p m b o -> p m (b o)"),
                         giq4[:, 8:12, :, t:t + 1].rearrange(
                             "p m b o -> p m (b o)"),
                         bnhhx_q, out_ap, tag="q")

            nslot = L * NCH
            slot = 0
            for t in range(L):
                x3full = k3(fxt[t], HQ)
                for c in range(NCH):
                    off = c * CH
                    h = hst[c]
                    h3 = k3(h, HQ)
                    x3 = x3full[:, :, off:off + CH]
                    prz = pp.tile([128, 8 * CH], F32, tag="frz")
                    for m in range(8):
                        tgt = prz[:, m * CH:(m + 1) * CH]
                        mm_dr(tgt, wf_ih3, m, x3, m in (0, 4))
                        mm_dr(tgt, wf_hh3, m, h3, False)
                    dr_inject(prz[:], brzx8, 0, 8 * CH)
                    rz = sp.tile([128, 8 * CH], BF16, tag="frzs")
                    nc.scalar.activation(rz[:], prz[:], AF.Sigmoid)

                    pnh = ppn.tile([128, 4 * CH], F32, tag="fn")
                    for mi in range(4):
                        mm_dr(pnh[:, mi * CH:(mi + 1) * CH], wf_hh3, 8 + mi,
                              h3, mi == 0)
                    dr_inject(pnh[:], bnhhx_fc, 0, 4 * CH)
                    t2 = sp.tile([128, 4 * CH], BF16, tag="ft2")
                    nc.vector.tensor_mul(t2[:], pnh[:], rz[:, 0:4 * CH])
                    pni = ppn.tile([128, 4 * CH], F32, tag="fn")
                    for mi in range(4):
                        mm_dr(pni[:, mi * CH:(mi + 1) * CH], wf_ih3, 8 + mi,
                              x3, mi == 0)
                    dr_inject(pni[:], bnihx_fc, 0, 4 * CH)
                    npre = sp.tile([128, 4 * CH], BF16, tag="fnp")
                    nc.vector.tensor_add(npre[:], t2[:], pni[:])
                    n_t = sp.tile([128, 4 * CH], BF16, tag="fnt")
                    nc.scalar.activation(n_t[:], npre[:], AF.Tanh)
                    d_t = sp.tile([128, 4 * CH], BF16, tag="fd")
                    nc.vector.tensor_sub(d_t[:], n_t[:], h[:])
                    u_t = sp.tile([128, 4 * CH], BF16, tag="fu")
                    nc.vector.tensor_mul(u_t[:], rz[:, 4 * CH:8 * CH], d_t[:])
                    if t == L - 1:
                        out_ap = k3(frepT, HQ)[:, :, off:off + CH]
                        nc.vector.tensor_add(out_ap, k3(h, HQ), k3(u_t, HQ))
                    else:
                        nc.vector.tensor_add(h[:], h[:], u_t[:])
                    slot += 1
                    while qstep[0] * nslot < slot * QL:
                        q_step()
            while qstep[0] < QL:
                q_step()

        # ============ episodic memory + answer + fc ============
        with tc.tile_pool(name="est", bufs=1) as stp, \
             tc.tile_pool(name="esp", bufs=2) as sp:
            w_a_ih = load(stp, w_a_ih_d, (128, HQ * G3), FP8)
            w_a_hh = load(stp, w_a_hh_d, (128, HQ * G3), FP8)
            w_m_ih = load(stp, w_m_ih_d, (128, HQ * G3), FP8)
            w_m_hh = load(stp, w_m_hh_d, (128, HQ * G3), FP8)
            g1t = load(stp, g1t_d, (128, 16 * H), FP8)
            g2t = load(stp, g2t_d, (128, HQ), FP8)
            w_ans_ih = load(stp, w_ans_ih_d, (128, 2 * HQ * G3), FP8)
            w_ans_hh = load(stp, w_ans_hh_d, (128, HQ * G3), FP8)
            wa_ih3, wa_hh3 = k3(w_a_ih, HQ), k3(w_a_hh, HQ)
            wm_ih3, wm_hh3 = k3(w_m_ih, HQ), k3(w_m_hh, HQ)
            wans_ih3, wans_hh3 = k3(w_ans_ih, 2 * HQ), k3(w_ans_hh, HQ)
            g1t3 = k3(g1t, 16)
            g2t3 = g2t[:].rearrange("p (k o) -> p k o", o=1)

            nc.vector.tensor_copy(memT[:], qrepT[:])
            dump(stp, "d_qrep", qrepT[:], (128, HQ * bn))
            dump(stp, "d_frep", frepT[:], (128, HQ * S))
            qexp = stp.tile([128, HQ * S], FP8, tag="qexp")
            nc.vector.tensor_copy(
                qexp[:].rearrange("p (qb f) -> p qb f", f=NF),
                qrepT[:].to_broadcast([128, HQ * bn, NF]))
            zfeat = stp.tile([128, 16 * S], FP8, tag="zfeat")
            mexp = stp.tile([128, HQ * S], FP8, tag="mexp")
            gia = stp.tile([128, MT * S], BF16, tag="gia")
            zsc = stp.tile([128, HQ * S], BF16, tag="zsc")
            relu = stp.tile([128, HQ * S], FP8, tag="relu")
            gexc = stp.tile([1, S], BF16, tag="gexc")
            gexb = stp.tile([128, S], BF16, tag="gexb")
            gexp = stp.tile([128, NF * HQ * bn], BF16, tag="gexp")
            gians = stp.tile([128, MT * bn], BF16, tag="gians")
            ansin = stp.tile([128, 2 * HQ * bn], FP8, tag="ansin")

            # episode-invariant zfeat halves: f*q and |f-q|
            nc.vector.tensor_mul(zfeat[:, 0:HQ * S], frepT[:], qexp[:])
            nc.vector.tensor_sub(zsc[:], frepT[:], qexp[:])
            nc.scalar.activation(zfeat[:, 2 * HQ * S:3 * HQ * S], zsc[:],
                                 AF.Abs)

            # attention-GRU input gates gia = Wih_a @ frep + bias, m-major
            with tc.tile_pool(name="giab", bufs=1) as gbp, \
                 tc.tile_pool(name="giap", bufs=2, space="PSUM") as gpp:
                giaxb = bias2(gbp, gib_a, MT, S, "giaxb")
                frep3 = k3(frepT, HQ)
                total = MT * S
                gt0 = 0
                while gt0 < total:
                    gw = min(1024, total - gt0)
                    psm = gpp.tile([128, 1024], F32, tag="giapp")
                    a0 = gt0
                    banks_opened = set()
                    while a0 < gt0 + gw:
                        m = a0 // S
                        a1 = min((m + 1) * S, gt0 + gw,
                                 256 * (a0 // 256 + 1))
                        bank = ((a0 - gt0) * 4) // 2048
                        op = bank not in banks_opened
                        banks_opened.add(bank)
                        tgt = psm[:, a0 - gt0:a1 - gt0]
                        for j in range(2):
                            nc.tensor.matmul(
                                tgt, wa_ih3[:, 2 * j:2 * j + 2,
                                            m * 128:(m + 1) * 128],
                                frep3[:, 2 * j:2 * j + 2,
                                      a0 - m * S:a1 - m * S],
                                start=op and j == 0, stop=False, perf_mode=DR,
                                skip_group_check=not (op and j == 0))
                        a0 = a1
                    dr_inject(psm[:, 0:gw], giaxb, gt0, gw)
                    nc.scalar.activation(gia[:, gt0:gt0 + gw],
                                         psm[:, 0:gw], AF.Identity)
                    gt0 += gw
            gia4 = gia[:].rearrange("p (m b f) -> p m b f", m=MT, b=bn)
            dump(stp, "d_gia", gia[:], (128, MT * S))

            # answer-GRU input gates (ans_in is constant across decode steps)
            nc.vector.tensor_copy(
                ansin[:, 0:HQ * bn].rearrange("p (q b) -> p q b", q=HQ),
                y0t[:].to_broadcast([128, HQ, bn]))
            nc.vector.tensor_copy(ansin[:, HQ * bn:2 * HQ * bn], qrepT[:])

            with tc.tile_pool(name="epz", bufs=2, space="PSUM") as pz, \
                 tc.tile_pool(name="epn", bufs=2, space="PSUM") as pn, \
                 tc.tile_pool(name="eg1", bufs=1, space="PSUM") as pg1, \
                 tc.tile_pool(name="eg2", bufs=2, space="PSUM") as pg2:
                # gians: all 12 m-tiles in one psum tile
                giansxb = bias2(stp, gib_ans, MT, bn, "giansxb")
                ans3 = k3(ansin, 2 * HQ)
                psm = pz.tile([128, MT * bn], F32, tag="rz")
                for m in range(MT):
                    tgt = psm[:, m * bn:(m + 1) * bn]
                    for j in range(4):
                        op = m == 0 and j == 0
                        nc.tensor.matmul(tgt, wans_ih3[:, 2 * j:2 * j + 2,
                                                       m * 128:(m + 1) * 128],
                                         ans3[:, 2 * j:2 * j + 2, :],
                                         start=op, stop=False,
                                         perf_mode=DR,
                                         skip_group_check=not op)
                dr_inject(psm[:], giansxb, 0, MT * bn)
                nc.scalar.activation(gians[:], psm[:], AF.Identity)
                gians3 = gians[:].rearrange("p (m b) -> p m b", m=MT)

                he = stp.tile([128, HQ * bn], FP8, tag="he")
                for e in range(EPISODES):
                    # memory-dependent zfeat halves
                    nc.vector.tensor_copy(
                        mexp[:].rearrange("p (qb f) -> p qb f", f=NF),
                        memT[:].to_broadcast([128, HQ * bn, NF]))
                    nc.vector.tensor_mul(zfeat[:, HQ * S:2 * HQ * S],
                                         frepT[:], mexp[:])
                    nc.vector.tensor_sub(zsc[:], frepT[:], mexp[:])
                    nc.scalar.activation(zfeat[:, 3 * HQ * S:4 * HQ * S],
                                         zsc[:], AF.Abs)
                    zf3 = k3(zfeat, 16)
                    # gate MLP layer 1: relu(zfeat @ g_w1.T + b1)
                    for m in range(HQ):
                        psg = pg1.tile([128, 1024], F32, tag="g1p")
                        b0 = 0
                        for nb in (256, 256, 128):
                            tgt = psg[:, b0:b0 + nb]
                            for j in range(8):
                                op = b0 in (0, 512) and j == 0
                                nc.tensor.matmul(
                                    tgt, g1t3[:, 2 * j:2 * j + 2,
                                              m * 128:(m + 1) * 128],
                                    zf3[:, 2 * j:2 * j + 2, b0:b0 + nb],
                                    start=op, stop=j == 7, perf_mode=DR,
                                    skip_group_check=not op)
                            b0 += nb
                        nc.scalar.activation(relu[:, m * S:(m + 1) * S],
                                             psg[:, 0:S], AF.Relu,
                                             bias=gb1[:, m:m + 1])
                    # gate MLP layer 2 + sigmoid
                    relu3 = k3(relu, HQ)
                    b0 = 0
                    for nb in (256, 256, 128):
                        psg2 = pg2.tile([1, 256], F32, tag="g2p")
                        for j in range(HQ):
                            nc.tensor.matmul(psg2[0:1, 0:nb],
                                             g2t[:, j:j + 1],
                                             relu3[:, j, b0:b0 + nb],
                                             start=j == 0, stop=j == HQ - 1)
                        nc.scalar.activation(gexc[0:1, b0:b0 + nb],
                                             psg2[0:1, 0:nb], AF.Sigmoid,
                                             bias=gb2[:])
                        b0 += nb
                    nc.gpsimd.partition_broadcast(gexb[:], gexc[0:1, :])
                    if e == 0:
                        dump(stp, "d_gex", gexb[:], (128, S))
                    # expand gates to (t, q, b) layout
                    nc.vector.tensor_copy(
                        gexp[:].rearrange("p (t q b) -> p t q b", t=NF, q=HQ),
                        gexb[:].rearrange("p (b t o) -> p t o b", o=1, b=bn)
                        .to_broadcast([128, NF, HQ, bn]))

                    nc.vector.memset(he[:], 0.0)
                    for t in range(NF):
                        gru_step(
                            sp, pz, pn, wa_hh3, he,
                            gia4[:, 0:8, :, t:t + 1].rearrange(
                                "p m b o -> p m (b o)"),
                            gia4[:, 8:12, :, t:t + 1].rearrange(
                                "p m b o -> p m (b o)"),
                            bnhhx_a, he[:],
                            gate=gexp[:, t * HQ * bn:(t + 1) * HQ * bn],
                            tag="e")
                    # memory GRU
                    gru_step(sp, pz, pn, wm_hh3, memT, None, None, bnhhx_m,
                             memT[:], ih=(wm_ih3, k3(he, HQ)), brzx2=brzx_m,
                             bnihx2=bnihx_m, tag="m")

                dump(stp, "d_mem", memT[:], (128, HQ * bn))
                # ---- answer decode ----
                hd4 = hdecT[:].rearrange("p (q b dd) -> p q b dd", q=HQ, b=bn)
                hcur = memT
                for di in range(nd):
                    gru_step(sp, pz, pn, wans_hh3, hcur,
                             gians3[:, 0:8, :], gians3[:, 8:12, :],
                             bnhhx_ans, hans[:], tag="a")
                    hcur = hans
                    nc.vector.tensor_copy(
                        hd4[:, :, :, di:di + 1],
                        hans[:].rearrange("p (q b) -> p q b",
                                          q=HQ).to_broadcast([128, HQ, bn, 1]))

        # ---- fc + log-softmax (two matmul passes) ----
        with tc.tile_pool(name="fcst", bufs=1) as stp, \
             tc.tile_pool(name="fcw", bufs=2) as fwp, \
             tc.tile_pool(name="fco", bufs=2) as fop, \
             tc.tile_pool(name="fcp", bufs=2, space="PSUM") as fpp:
                hdec3 = k3(hdecT, HQ)
                onez3 = onez[0:1, :].rearrange("p (k m) -> p k m", k=2)
                sums = stp.tile([nv, len(VSLABS)], F32, tag="sums")

                def slab_tile(si, eng):
                    t, soff, vs = fct_sb[si]
                    if t is not None:
                        return t
                    st = fwp.tile([128, HQ * VSLAB], FP8, tag="fcts")
                    eng.dma_start(st[:, 0:HQ * vs],
                                  fct_d[:, soff:soff + HQ * vs])
                    return st

                def fc_mms(si, wt, eng):
                    _, soff, vs = fct_sb[si]
                    voff = soff // HQ
                    psm = fpp.tile([nv, VSLAB], F32, tag="fcp")
                    w3 = wt[:, 0:HQ * vs].rearrange("p (k v) -> p k v", k=HQ)
                    nsb = (vs + 255) // 256
                    for i in range(nsb):
                        i0, i1 = i * 256, min((i + 1) * 256, vs)
                        tgt = psm[:, i0:i1]
                        for j in range(2):
                            op = i % 2 == 0 and j == 0
                            nc.tensor.matmul(tgt, hdec3[:, 2 * j:2 * j + 2, :],
                                             w3[:, 2 * j:2 * j + 2, i0:i1],
                                             start=op, stop=False,
                                             perf_mode=DR,
                                             skip_group_check=not op)
                    fcbs = fop.tile([1, 2 * VSLAB], FP8, tag="fcbs")
                    eng.dma_start(fcbs[0:1, 0:2 * vs],
                                  fcb2_d[0:1, 2 * voff:2 * voff + 2 * vs])
                    for i in range(nsb):
                        i0, i1 = i * 256, min((i + 1) * 256, vs)
                        nc.tensor.matmul(psm[:, i0:i1], onez[0:1, 0:nv],
                                         fcbs[0:1, i0:i1], start=False,
                                         stop=True, skip_group_check=True)
                    return psm, vs, voff

                for si in range(len(VSLABS)):
                    wt = slab_tile(si, nc.gpsimd)
                    psm, vs, voff = fc_mms(si, wt, nc.gpsimd)
                    ex = fop.tile([nv, VSLAB], BF16, tag="ex")
                    nc.scalar.activation(ex[:, 0:vs], psm[:, 0:vs], AF.Exp,
                                         accum_out=sums[:, si:si + 1])
                ssum = stp.tile([nv, 1], F32, tag="ssum")
                nc.vector.reduce_sum(ssum[:], sums[:],
                                     axis=mybir.AxisListType.X)
                logz = stp.tile([nv, 1], F32, tag="logz")
                nc.scalar.activation(logz[:], ssum[:], AF.Ln)
                nlz = stp.tile([nv, 1], F32, tag="nlz")
                nc.vector.tensor_scalar(nlz[:], logz[:], -1.0, None, ALU.mult)

                for si in range(len(VSLABS)):
                    wt = slab_tile(si, nc.sync)
                    psm, vs, voff = fc_mms(si, wt, nc.sync)
                    stage = fop.tile([nv, VSLAB], BF16, tag="stage")
                    nc.scalar.activation(stage[:, 0:vs], psm[:, 0:vs],
                                         AF.Identity, bias=nlz[:, 0:1])
                    nc.sync.dma_start(out_d[:, voff:voff + vs],
                                      stage[:, 0:vs])

    nc.compile()
    return nc


def host_prep(inputs, cfg: Cfg):
    bc, nd = cfg.bc, cfg.nd
    emb = np.asarray(inputs["emb"], np.float32).copy()
    emb[0] = 0.0
    facts = np.asarray(inputs["facts"])
    questions = np.asarray(inputs["questions"])
    b = facts.shape[0]
    ncores = b // bc

    flens = (np.asarray(inputs["facts_mask"]).reshape(b * NF, L) == 0).sum(-1)
    qlens = (np.asarray(inputs["question_masks"]) == 0).sum(-1)
    assert (flens == L).all() and (qlens == QL).all(), \
        "kernel requires full-length sequences (masks all zero)"

    ii = {k: np.asarray(vv, np.float32) for k, vv in inputs.items()
          if k not in ("facts", "facts_mask", "questions", "question_masks",
                       "num_decode")}

    def negz(w):
        w = w.copy()
        w[H:2 * H] *= -1.0
        return w

    def wt_tiles(w, kt):
        wt = w.T.reshape(kt, 128, w.shape[0]).transpose(1, 0, 2)
        return np.ascontiguousarray(wt).reshape(128, kt * w.shape[0]).astype(f8)

    def col_tiles(x, ncol):
        return np.ascontiguousarray(x.reshape(ncol, 128).T).astype(np.float32)

    shared = {}
    for nm, wih, whh in (("f", "ig_Wih", "ig_Whh"), ("q", "qg_Wih", "qg_Whh"),
                         ("a", "a_Wih", "a_Whh"), ("m", "m_Wih", "m_Whh")):
        shared[f"w_{nm}_ih"] = wt_tiles(negz(ii[wih]), HQ)
        shared[f"w_{nm}_hh"] = wt_tiles(negz(ii[whh]), HQ)
    shared["w_ans_ih"] = wt_tiles(negz(ii["ans_Wih"]), 2 * HQ)
    shared["w_ans_hh"] = wt_tiles(negz(ii["ans_Whh"]), HQ)
    g1 = ii["g_w1"].T  # (4H, H)
    shared["g1t"] = np.ascontiguousarray(
        g1.reshape(16, 128, H).transpose(1, 0, 2)).reshape(
            128, 16 * H).astype(f8)
    shared["g2t"] = col_tiles(ii["g_w2"][0], HQ).astype(f8)

    fcw = ii["fc_w"]
    fct_full = np.ascontiguousarray(
        fcw.T.reshape(HQ, 128, V).transpose(1, 0, 2))  # (128, HQ, V)
    slabs = []
    off = 0
    for vs in VSLABS:
        slabs.append(fct_full[:, :, off:off + vs].reshape(128, HQ * vs))
        off += vs
    shared["fct"] = np.concatenate(slabs, axis=1).astype(f8)
    fcb = ii["fc_b"]
    fcb2 = []
    off = 0
    for vs in VSLABS:
        fcb2.append(fcb[off:off + vs])
        fcb2.append(np.zeros(vs, np.float32))
        off += vs
    shared["fcb2"] = np.concatenate(fcb2).reshape(1, 2 * V).astype(f8)
    shared["y0t"] = col_tiles(emb[1], HQ).astype(f8)
    shared["i128"] = np.eye(128, dtype=np.float32).astype(bf16)
    i2 = np.zeros((128, 256), np.float32)
    i2[:, 0:128] = np.eye(128)
    shared["i2f8"] = i2.astype(f8)
    onez = np.zeros((1, 128), np.float32)
    onez[0, 0:cfg.nv] = 1.0
    shared["onez"] = onez.astype(f8)

    def neg_rz(v):
        v = v.copy()
        v[H:2 * H] *= -1.0
        return v

    for nm, bih, bhh in (("f", "ig_bih", "ig_bhh"), ("q", "qg_bih", "qg_bhh"),
                         ("a", "a_bih", "a_bhh"), ("m", "m_bih", "m_bhh"),
                         ("ans", "ans_bih", "ans_bhh")):
        bi, bh = ii[bih], ii[bhh]
        if nm in ("q", "a", "ans"):
            gib = np.concatenate([neg_rz(bi + bh)[0:2 * H], bi[2 * H:3 * H]])
            shared[f"gib_{nm}"] = col_tiles(gib, MT)
        shared[f"bnhh_{nm}"] = col_tiles(bh[2 * H:3 * H], 4)
        if nm in ("f", "m"):
            shared[f"brz_{nm}"] = col_tiles(neg_rz(bi + bh)[0:2 * H], 8)
            shared[f"bnih_{nm}"] = col_tiles(bi[2 * H:3 * H], 4)
    shared["gb1"] = col_tiles(ii["g_b1"], HQ)
    shared["gb2"] = ii["g_b2"].reshape(1, 1).astype(np.float32)

    # inputs: pre-gathered + pre-transposed, fp8
    fx = emb[facts.reshape(-1)].reshape(b * NF, L, HQ, 128).astype(f8)
    qx = emb[questions.reshape(-1)].reshape(b, QL, HQ, 128).astype(f8)

    in_maps = []
    for c in range(ncores):
        m = dict(shared)
        a = fx[c * bc * NF:(c + 1) * bc * NF]          # (S, L, HQ, 128)
        m["fxT"] = np.ascontiguousarray(
            a.transpose(1, 3, 2, 0)).reshape(L * 128, HQ * S)
        qa = qx[c * bc:(c + 1) * bc].reshape(bc * QL, HQ, 128)
        m["qxT"] = np.ascontiguousarray(
            qa.transpose(2, 1, 0)).reshape(128, HQ * NTQ)
        in_maps.append(m)
    return in_maps


def kernel(**inputs):
    nd = int(np.asarray(inputs["num_decode"]))
    cfg = Cfg(nd=nd)
    if cfg.key not in _COMPILED:
        _COMPILED[cfg.key] = build(cfg)
    nc = _COMPILED[cfg.key]
    in_maps = host_prep(inputs, cfg)
    res = bass_utils.run_bass_kernel_spmd(nc, in_maps,
                                          core_ids=list(range(N_CORES)))
    out = np.concatenate([res.results[c]["out"] for c in range(N_CORES)], 0)
    return np.ascontiguousarray(out.astype(np.float32))
